# revision 1
# baseline (speedup 1.0000x reference)
"""MultiHeadChannelAttention Bass kernel for 8 Trainium2 NeuronCores.

Problem (hardcoded shapes): x (2, 512, 64, 32) fp32; Wq/Wk/Wv/Wfc (512, 512);
biases (512,). Reference math per batch b, with X = x[b].reshape(2048, 512):
  Q = X Wq^T + bq ; K = X Wk^T + bk ; V = X Wv^T + bv   (heads of 64 dims)
  out = softmax(QK^T/8) V  (per head), concat heads, @ Wfc^T + bfc

Sharding: 8 cores = 2 batches x 4 token-blocks of 512 tokens. Each core
computes K/V for all 2048 tokens of its batch (4x redundant), Q/attention/fc
only for its 512-token block. No cross-core communication; the host only
slices inputs and concatenates outputs. Tokens are rotated per-core so the
core's own block sits at columns 0:512 of X^T — the Q projection then reads
the same xt tiles as K/V and no separate xq tensor is shipped.

Device layouts (all matmul-friendly, weights pre-transposed on host):
  XT  [512c, 2048t]  = X^T (rotated) in two column-half tiles per chunk
  KT [512, 2048] = (Wk X^T + bk);  QT [512, 512]
  scoresT [j, i] per head via row-tiled K=64 matmul pairs (2 heads/PE pass,
  concurrent in the PE array via distinct row groups)
  exp on ScalarE from 2-bank PSUM; on ~1/4 of tiles a DVE
  Schraudolph fast-exp (bf16-bit-pattern trick, one TENSOR_SCALAR) stands in
  so the exp stream isn't ScalarE-paced; attnV with ones-column (M=65) so the
  softmax denominator falls out of the same matmul; fc consumes attnout^T
  directly. bv is folded into the fc bias on host (softmax rows sum to 1).
  Softmax reciprocal = exp(-ln(x)) on ScalarE, batched [2, 512] per pair.
"""

import numpy as np
import ml_dtypes

N_CORES = 8
B, C, N_TOK, TB = 2, 512, 2048, 512
HEADS, DK = 8, 64
NCH = C // 128  # channel chunks (4)
NJT = N_TOK // 128  # key-token tiles (16)
NTT = TB // 128  # fc token tiles (4)
HT = N_TOK // 2  # xt column-half (1024)

# Schraudolph fast-exp constants for bf16 bit patterns, including the 0.125
# attention scale: i16 = round(score * (0.125*128/ln2) + (127*128 - 5.5))
FEXP_C1 = 0.125 * 128.0 / float(np.log(2.0))
FEXP_C2 = 127.0 * 128.0 - 5.25
# iterations (per head-pair) whose exp runs on DVE instead of ScalarE
FEXP_J = {0: (), 1: (1, 7, 13), 2: (1, 7, 13), 3: (1, 3, 5, 9, 11, 13)}

_CACHE = {}


def _install_tile_drain_patch():
    """The end-of-kernel Tile drain can carry several sem waits; this
    walrus build allows one wait per non-EVSEM instruction. Split the
    waits across a chain of drains."""
    import bass_rust
    from concourse import tile as _tile
    from concourse.vector_clock import ScopedClock

    if getattr(_tile.TileContext, "_drain_patch_installed", False):
        return

    def _patched(self, tick_clock, wait_clock):
        nc = self.nc
        drain_inst = nc.sync.drain()
        wait_clock.add_sem_waits(
            drain_inst.ins, ScopedClock({None: tick_clock.global_clock})
        )
        si = drain_inst.ins.sync_info
        if si is not None and len(si.on_wait) > 1:
            waits = list(si.on_wait)
            drain_inst.ins.sync_info = bass_rust.SyncInfo(
                on_wait=[waits[0]], on_update=list(si.on_update)
            )
            for w in waits[1:]:
                extra = nc.sync.drain()
                extra.ins.sync_info = bass_rust.SyncInfo(on_wait=[w], on_update=[])
        nc.all_engine_barrier()
        assert self.sems is not None
        popped = nc._tile_sem_poison_stack.pop()
        assert popped is self._sem_poison
        nc.clear_and_free_semaphores(list(self.sems.allocated().values()))
        nc.all_engine_barrier()

    _tile.TileContext._drain_and_barrier = _patched
    _tile.TileContext._drain_patch_installed = True


def _split_multi_waits(nc):
    """This walrus build accepts one sync wait per instruction (two on
    EVSEM). Tile can attach two; move extras onto preceding NOPs."""
    import concourse.mybir as mybir

    for f in nc.m.functions:
        for bb in f.blocks:
            out = []
            changed = False
            for ins in bb.instructions:
                si = ins.sync_info
                limit = 2 if isinstance(ins, mybir.InstEventSemaphore) else 1
                if si is not None and len(si.on_wait) > limit:
                    waits = list(si.on_wait)
                    keep = waits[-limit:]
                    for i, w in enumerate(waits[:-limit]):
                        nop = mybir.InstNoOp(
                            name=f"{ins.name}_w{i}",
                            engine=ins.engine,
                            sync_info=mybir.SyncInfo(on_wait=[w], on_update=[]),
                            bass_nofuse=True,
                        )
                        nc.register_instruction(nop, overwrite=True)
                        out.append(nop)
                    ins.sync_info = mybir.SyncInfo(
                        on_wait=keep, on_update=list(si.on_update)
                    )
                    changed = True
                out.append(ins)
            if changed:
                bb.instructions = out


def _build():
    import concourse.bass as bass
    import concourse.mybir as mybir
    import concourse.tile as tile
    from concourse.bass import ts

    dt = mybir.dt
    f32, bf16, i16 = dt.float32, dt.bfloat16, dt.int16
    Exp = mybir.ActivationFunctionType.Exp
    Ln = mybir.ActivationFunctionType.Ln
    Mult, Add = mybir.AluOpType.mult, mybir.AluOpType.add

    nc = bass.Bass()
    # weights are host-interleaved to [128, NCH*cols] so each DMA moves
    # one big per-partition span (large DMA packets) while chunk c still
    # slices out as [:, c*cols : ...] with partition p = channel 128c+p
    xt_d = nc.dram_tensor("xt", [C, N_TOK], bf16, kind="ExternalInput")
    wqT_d = nc.dram_tensor("wqT", [128, NCH * C], bf16, kind="ExternalInput")
    wkT_d = nc.dram_tensor("wkT", [128, NCH * C], bf16, kind="ExternalInput")
    wvT_d = nc.dram_tensor("wvT", [128, NCH * C], bf16, kind="ExternalInput")
    wfT_d = nc.dram_tensor("wfT", [128, NCH * C], bf16, kind="ExternalInput")
    bias_d = nc.dram_tensor("bias", [128, 2 * NCH], f32, kind="ExternalInput")
    bfc_d = nc.dram_tensor("bfc", [1, C], bf16, kind="ExternalInput")
    out_d = nc.dram_tensor("out", [TB, C], f32, kind="ExternalOutput")

    with tile.TileContext(nc) as tc:
        with (
            tc.tile_pool(name="wp", bufs=1) as wp,
            tc.tile_pool(name="data", bufs=1) as data,
            tc.tile_pool(name="ep", bufs=6) as ep,
            tc.tile_pool(name="np_", bufs=2) as npool,
            tc.tile_pool(name="scp", bufs=2, space=bass.MemorySpace.PSUM) as scp,
            tc.tile_pool(name="ap_", bufs=1, space=bass.MemorySpace.PSUM) as apool,
            tc.tile_pool(name="aux", bufs=2, space=bass.MemorySpace.PSUM) as aux,
        ):
            # ---- constants / weights (merged [128, NCH*cols] tiles) ----
            wq_all = wp.tile([128, NCH * C], bf16, tag="wq", name="wq_all")
            wk_all = wp.tile([128, NCH * C], bf16, tag="wk", name="wk_all")
            wv_all = wp.tile([128, NCH * C], bf16, tag="wv", name="wv_all")
            wf_all = wp.tile([128, NCH * C], bf16, tag="wf", name="wf_all")
            wq = [wq_all[:, ts(c, C)] for c in range(NCH)]
            wk = [wk_all[:, ts(c, C)] for c in range(NCH)]
            wv = [wv_all[:, ts(c, C)] for c in range(NCH)]
            wf = [wf_all[:, ts(c, C)] for c in range(NCH)]
            bias_all = wp.tile([128, 2 * NCH], f32, tag="bias", name="bias_all")
            bqt = [bias_all[:, d : d + 1] for d in range(NCH)]
            bkt = [bias_all[:, NCH + d : NCH + d + 1] for d in range(NCH)]
            bfct = wp.tile([1, C], bf16, tag="bfct", name="bfct")
            ones_t = wp.tile([128, TB], bf16, tag="ones", name="ones_t")
            nc.gpsimd.memset(ones_t[:], 1.0)
            ones_f = wp.tile([128, 64], f32, tag="onesf", name="ones_f")
            nc.vector.memset(ones_f[:], 1.0)

            # ---- activations in: two column-half tiles per channel chunk ----
            xta = [
                data.tile([128, HT], bf16, tag=f"xta{c}", name=f"xta{c}")
                for c in range(NCH)
            ]
            xtb = [
                data.tile([128, HT], bf16, tag=f"xtb{c}", name=f"xtb{c}")
                for c in range(NCH)
            ]

            def xk(jb):  # xt tile + column block for K-proj token block jb
                return (xta if jb < 2 else xtb), jb % 2

            def xv(j):  # xt tile + column tile for V-proj token tile j
                return (xta if j < 8 else xtb), j % 8

            # ---- input DMAs over the three issue paths (SP/ACT HWDGE +
            # gpsimd SWDGE). Weights lead on scalar+gpsimd; the xt first
            # halves (needed by Q proj and the early K/V token blocks)
            # lead on sync ----
            nc.scalar.dma_start(out=wq_all[:], in_=wqT_d[:])
            nc.gpsimd.dma_start(out=wk_all[:], in_=wkT_d[:])
            nc.sync.dma_start(out=xta[0][:], in_=xt_d[ts(0, 128), 0:HT])
            nc.sync.dma_start(out=xta[1][:], in_=xt_d[ts(1, 128), 0:HT])
            nc.sync.dma_start(out=xta[2][:], in_=xt_d[ts(2, 128), 0:HT])
            nc.sync.dma_start(out=xta[3][:], in_=xt_d[ts(3, 128), 0:HT])
            nc.scalar.dma_start(out=wv_all[:], in_=wvT_d[:])
            nc.gpsimd.dma_start(out=xtb[0][:], in_=xt_d[ts(0, 128), HT:N_TOK])
            nc.scalar.dma_start(out=xtb[1][:], in_=xt_d[ts(1, 128), HT:N_TOK])
            nc.gpsimd.dma_start(out=xtb[2][:], in_=xt_d[ts(2, 128), HT:N_TOK])
            nc.scalar.dma_start(out=xtb[3][:], in_=xt_d[ts(3, 128), HT:N_TOK])
            nc.sync.dma_start(out=bias_all[:], in_=bias_d[:])
            nc.gpsimd.dma_start(out=wf_all[:], in_=wfT_d[:])
            nc.sync.dma_start(out=bfct[:], in_=bfc_d[:])

            # trigger the natural_log_exp ACT table load during the DMA
            # window instead of right before the first real exp
            tbl = npool.tile([1, 64], f32, tag="tbl", bufs=1, name="tbl")
            nc.scalar.activation(out=tbl[:], in_=ones_f[0:1, :], func=Ln)

            # PE warmup: one dummy accumulation chain on the ones tile keeps
            # the HAM activity monitor busy through the input-load window so
            # the first real projections run at 2.4 GHz
            warm = aux.tile([128, TB], f32, tag="aux", name="warm")
            for r in range(10):
                nc.tensor.matmul(
                    warm[:], ones_t[0:1, 0:128], ones_t[0:1, :],
                    start=(r == 0), stop=(r == 9),
                )

            # ---- persistent intermediates ----
            kt = [data.tile([128, N_TOK], bf16, tag=f"kt{d}", name=f"kt{d}") for d in range(NCH)]
            qt = [data.tile([128, TB], bf16, tag=f"qt{d}", name=f"qt{d}") for d in range(NCH)]
            vpad = [
                data.tile([128, HEADS, DK + 1], bf16, tag=f"vp{j}", name=f"vp{j}")
                for j in range(NJT)
            ]
            att = [
                data.tile([128, TB], bf16, tag=f"att{c}", name=f"att{c}")
                for c in range(NCH)
            ]

            def proj_q(d):
                """Q^T d-tile (128 chans = heads 2d, 2d+1) + bias."""
                qp = aux.tile([128, TB], f32, tag="aux", name=f"qp{d}")
                for c in range(NCH):
                    nc.tensor.matmul(
                        qp[:], wq[c][:, ts(d, 128)], xta[c][:, 0:TB],
                        start=(c == 0), stop=(c == NCH - 1),
                    )
                nc.vector.tensor_scalar_add(out=qt[d][:], in0=qp[:], scalar1=bqt[d][:])

            def proj_k(d, jb):
                """K^T d-tile, token block jb + bias."""
                xt_half, hb = xk(jb)
                kp = aux.tile([128, TB], f32, tag="aux", name=f"kp{d}_{jb}")
                for c in range(NCH):
                    nc.tensor.matmul(
                        kp[:], wk[c][:, ts(d, 128)], xt_half[c][:, ts(hb, TB)],
                        start=(c == 0), stop=(c == NCH - 1),
                    )
                nc.vector.tensor_scalar_add(
                    out=kt[d][:, ts(jb, TB)], in0=kp[:], scalar1=bkt[d][:]
                )

            def proj_kq(d):
                proj_q(d)
                for jb in range(N_TOK // TB):
                    proj_k(d, jb)

            def proj_v(j):
                """V j-tile -> padded [128, 8, 65] with ones in column 64.
                The PSUM->SBUF evacuations alternate between ScalarE and
                DVE so neither engine paces pair 0."""
                xt_half, hj = xv(j)
                vp = aux.tile([128, C], f32, tag="aux", name=f"vpp{j}")
                for c in range(NCH):
                    nc.tensor.matmul(
                        vp[:], xt_half[c][:, ts(hj, 128)], wv[c][:],
                        start=(c == 0), stop=(c == NCH - 1),
                    )
                src = vp[:].rearrange("p (h d) -> p h d", h=HEADS)
                if j % 2 == 0:
                    nc.scalar.copy(out=vpad[j][:, :, 0:DK], in_=src)
                else:
                    nc.vector.tensor_copy(out=vpad[j][:, :, 0:DK], in_=src)
                nc.vector.memset(vpad[j][:, :, DK : DK + 1], 1.0)

            # ---- main pipeline ----
            def norm_gather(pp, a_sb0, a_sb1):
                """Collect the pair's two softmax denominators into one
                tile (rows 0 and 32 — matmul rhs base partitions must be
                0/32/64) so the ln/exp reciprocal runs as one ScalarE call
                per function instead of two. Rows 1-31 carry garbage that
                nothing reads."""
                dn = npool.tile([33, TB], f32, tag="dn", bufs=2, name=f"dn{pp}")
                nc.vector.tensor_copy(out=dn[0:1, :], in_=a_sb0[64:65, :])
                nc.vector.tensor_copy(out=dn[32:33, :], in_=a_sb1[64:65, :])
                return dn

            def norm_recip(pp, dn):
                """Batched reciprocal of both denominators on ScalarE as
                exp(-ln(x)) — both functions live in one ACT table set, and
                it keeps the slow iterative divide off DVE."""
                lnt = npool.tile([33, TB], f32, tag="lnt", bufs=2, name=f"lnt{pp}")
                nc.scalar.activation(out=lnt[:], in_=dn[:], func=Ln)
                rcp = npool.tile([33, TB], f32, tag="rcp", bufs=2, name=f"rcp{pp}")
                nc.scalar.activation(out=rcp[:], in_=lnt[:], func=Exp, scale=-1.0)
                return rcp

            def norm_apply(pp, hh, a_sb, rcp, rb_pool=None, rb_tag="aux"):
                rb_pool = aux if rb_pool is None else rb_pool
                rb = rb_pool.tile([64, TB], f32, tag=rb_tag, name=f"rb{pp}_{hh}")
                r = 32 * hh
                nc.tensor.matmul(rb[:], ones_f[r : r + 1, :], rcp[r : r + 1, :])
                nc.vector.tensor_mul(
                    out=att[pp][ts(hh, 64), :], in0=a_sb[0:64, :], in1=rb[:]
                )

            proj_q(0)
            proj_k(0, 0)
            fps = []  # fc PSUM accumulators; t=0/1 filled in pair 3's loop
            prev = None  # previous pair's SBUF accumulator copies
            prev_dn = None
            prev_rcp = None
            for p in range(NCH):  # head pair p = heads 2p, 2p+1
                a0 = apool.tile([DK + 1, TB], f32, tag="a0", name=f"a0_{p}")
                a1 = apool.tile([DK + 1, TB], f32, tag="a1", name=f"a1_{p}")
                fexp_js = FEXP_J[p]

                def attn_v(j, e):
                    nc.tensor.matmul(
                        a0[:], vpad[j][:, 2 * p, :], e[:, 0:TB],
                        start=(j == 0), stop=(j == NJT - 1),
                    )
                    nc.tensor.matmul(
                        a1[:], vpad[j][:, 2 * p + 1, :], e[:, TB : 2 * TB],
                        start=(j == 0), stop=(j == NJT - 1),
                    )

                pend = None  # (j, e) whose attnV is deferred one iteration
                for j in range(NJT):
                    # pair 0: the rest of K^T, paced with the xt DMA stream
                    # (jb 2/3 need the xt second halves, which land late)
                    if p == 0 and j in (1, 4, 6):
                        proj_k(0, {1: 1, 4: 2, 6: 3}[j])
                    # next pair's K/Q projection: the early pieces (needed
                    # by its first scores) run mid-pair; the late jb pieces
                    # are emitted at the boundary below as PE filler.
                    # Pair 0 already carries the V projection, so all of
                    # pair 1's pieces move to the boundary instead.
                    if 0 < p < NCH - 1:
                        if j == 10:
                            proj_q(p + 1)
                        elif j in (12, 14):
                            proj_k(p + 1, (j - 12) // 2)
                    # previous pair's normalization, deferred into this
                    # pair's loop so its reciprocal/broadcast work doesn't
                    # gate PE at the boundary. The last pair normalizes
                    # early (j=0-2) so att[2] is ready for in-loop fc
                    # prefill — real PE work that keeps the clock monitor
                    # from throttling the otherwise projection-free pair 3.
                    if prev is not None:
                        if p < NCH - 1:
                            if j == 3:
                                prev_rcp = norm_recip(p - 1, prev_dn)
                            elif j in (5, 7):
                                hh = int(j == 7)
                                norm_apply(p - 1, hh, prev[hh], prev_rcp)
                        else:
                            if j == 0:
                                prev_rcp = norm_recip(p - 1, prev_dn)
                            elif j in (1, 2):
                                hh = j - 1
                                norm_apply(p - 1, hh, prev[hh], prev_rcp)
                    # fc prefill for token chunks 0/1 interleaved into the
                    # last pair's loop (aux PSUM slots are free here). These
                    # full-array matmuls also re-warm the clock monitor,
                    # which the half-array sc/attnV mix cannot.
                    if p == NCH - 1 and j in (3, 5, 7, 9, 11, 13):
                        t = int(j >= 9)
                        c = ((j - 3) % 6) // 2
                        if c == 0:
                            fp = aux.tile([128, C], f32, tag="aux", name=f"fp{t}")
                            fps.append(fp)
                            nc.tensor.matmul(
                                fp[:], ones_t[0:1, 0:128], bfct[:],
                                start=True, stop=False,
                            )
                        nc.tensor.matmul(
                            fps[t][:], att[c][:, ts(t, 128)], wf[c][:],
                            start=False, stop=False,
                        )
                    sc = scp.tile([128, 2 * TB], f32, tag="sc", name=f"sc{p}_{j}")
                    nc.tensor.matmul(
                        sc[:, 0:TB], kt[p][0:64, ts(j, 128)], qt[p][0:64, :]
                    )
                    nc.tensor.matmul(
                        sc[:, TB : 2 * TB], kt[p][64:128, ts(j, 128)], qt[p][64:128, :]
                    )
                    e = ep.tile([128, 2 * TB], bf16, tag="e", name=f"e{p}_{j}")
                    if j in fexp_js:
                        # Schraudolph fast-exp on DVE: bf16 bit pattern via
                        # int16 round(x*C1 + C2); error ~3% per weight,
                        # washes out over the 2048-key softmax average
                        nc.vector.tensor_scalar(
                            out=e[:].bitcast(i16), in0=sc[:],
                            scalar1=FEXP_C1, scalar2=FEXP_C2, op0=Mult, op1=Add,
                        )
                    else:
                        nc.scalar.activation(out=e[:], in_=sc[:], func=Exp, scale=0.125)
                    # V projection emitted after scores/exp so a late wv/xt
                    # DMA can't block the PE stream ahead of the scores
                    if p == 0:
                        proj_v(j)
                    # attnV deferred one iteration: the tensor queue is
                    # strict FIFO, so attnV(j) right here would block the
                    # already-queued sc(j+1) behind exp(j). One iteration
                    # of skew keeps PE from ever waiting on the exp stream.
                    if pend is not None:
                        attn_v(*pend)
                    pend = (j, e)
                attn_v(*pend)
                # evacuate accumulators to SBUF (frees the PSUM banks for
                # the next pair while normalization runs off the copies)
                a_sb0 = npool.tile([DK + 1, TB], f32, tag="asb", bufs=4, name=f"asb0_{p}")
                a_sb1 = npool.tile([DK + 1, TB], f32, tag="asb", bufs=4, name=f"asb1_{p}")
                nc.vector.tensor_copy(out=a_sb0[:], in_=a0[:])
                nc.scalar.copy(out=a_sb1[:], in_=a1[:])
                prev = (a_sb0, a_sb1)
                prev_dn = norm_gather(p, a_sb0, a_sb1)
                # boundary PE filler: the next pair's late K pieces (not
                # needed until its scores j>=8) keep HAM warm through the
                # cross-pair dependency funnel
                if p == 0:
                    proj_q(1)
                    proj_k(1, 0)
                    proj_k(1, 1)
                if p + 1 < NCH:
                    proj_k(p + 1, 2)
                    proj_k(p + 1, 3)

            # ---- tail: all four fc tiles pre-accumulate bias + the first
            # three chunks (PE filler while the last pair's reciprocals run
            # on ScalarE); only the final chunk waits on normalize(3) ----
            def fc_prefill(t, fp):
                nc.tensor.matmul(
                    fp[:], ones_t[0:1, 0:128], bfct[:], start=True, stop=False
                )
                for c in range(NCH - 1):
                    nc.tensor.matmul(
                        fp[:], att[c][:, ts(t, 128)], wf[c][:],
                        start=False, stop=False,
                    )

            for t in (2, 3):
                # scores pool is draining by now; reuse its slots
                fp = scp.tile([128, C], f32, tag="sc", name=f"fp{t}")
                fc_prefill(t, fp)
                fps.append(fp)
            # last pair's normalization; rb goes in npool-independent spare
            # (scores slots are taken by fp2/fp3, aux by fp0/fp1) — use the
            # attnV accumulator pool, which is free after the acopies.
            # The att multiplies are split by fc token chunk so each fc
            # final matmul starts as soon as its slice is normalized.
            rcp3 = norm_recip(NCH - 1, prev_dn)
            rb3 = []
            for hh in range(2):
                rb = apool.tile([64, TB], f32, tag=("a0", "a1")[hh], name=f"rb3_{hh}")
                r = 32 * hh
                nc.tensor.matmul(rb[:], ones_f[r : r + 1, :], rcp3[r : r + 1, :])
                rb3.append(rb)

            out_q = [nc.sync, nc.scalar, nc.gpsimd, nc.sync]
            for t in range(NTT):
                for hh in range(2):
                    nc.vector.tensor_mul(
                        out=att[NCH - 1][ts(hh, 64), ts(t, 128)],
                        in0=prev[hh][0:64, ts(t, 128)],
                        in1=rb3[hh][:, ts(t, 128)],
                    )
                nc.tensor.matmul(
                    fps[t][:], att[NCH - 1][:, ts(t, 128)], wf[NCH - 1][:],
                    start=False, stop=True,
                )
                ot = npool.tile([128, C], f32, tag="ot", bufs=4, name=f"ot{t}")
                nc.vector.tensor_copy(out=ot[:], in_=fps[t][:])
                out_q[t].dma_start(out=out_d[ts(t, 128), :], in_=ot[:])

    _split_multi_waits(nc)
    nc.finalize()
    return nc


def get_nc():
    if "nc" not in _CACHE:
        _install_tile_drain_patch()
        _CACHE["nc"] = _build()
    return _CACHE["nc"]


def make_in_maps(x, Wq, bq, Wk, bk, Wv, bv, Wfc, bfc):
    bf = ml_dtypes.bfloat16
    x = np.asarray(x, np.float32)
    Wq, Wk, Wv, Wfc = (np.asarray(w, np.float32) for w in (Wq, Wk, Wv, Wfc))
    bq, bk, bv, bfc = (np.asarray(v, np.float32) for v in (bq, bk, bv, bfc))

    def interleave(wT):
        # [C, cols] -> [128, NCH*cols] with chunk c at columns [c*cols:...]
        cols = wT.shape[1]
        return np.ascontiguousarray(
            wT.reshape(NCH, 128, cols).transpose(1, 0, 2).reshape(128, NCH * cols)
        )

    bfc_folded = (Wfc @ bv + bfc).reshape(1, C).astype(bf)
    wqT = interleave(np.ascontiguousarray(Wq.T).astype(bf))
    wkT = interleave(np.ascontiguousarray(Wk.T).astype(bf))
    wvT = interleave(np.ascontiguousarray(Wv.T).astype(bf))
    wfT = interleave(np.ascontiguousarray(Wfc.T).astype(bf))
    bias_c = np.concatenate(
        [bq.reshape(NCH, 128).T, bk.reshape(NCH, 128).T], axis=1
    ).astype(np.float32)

    in_maps = []
    for core in range(N_CORES):
        b, t = divmod(core, N_TOK // TB)
        XT = np.ascontiguousarray(x[b].reshape(N_TOK, C).T).astype(bf)
        # rotate tokens so this core's block sits at columns 0:TB — the Q
        # projection then reads xt directly (attention is j-permutation
        # invariant, so K/V token order doesn't matter as long as kt and
        # vpad agree, which they do: both derive from this xt)
        perm = np.r_[t * TB : (t + 1) * TB, 0 : t * TB, (t + 1) * TB : N_TOK]
        in_maps.append(
            {
                "xt": np.ascontiguousarray(XT[:, perm]),
                "wqT": wqT,
                "wkT": wkT,
                "wvT": wvT,
                "wfT": wfT,
                "bias": bias_c,
                "bfc": bfc_folded,
            }
        )
    return in_maps


def assemble(outs):
    """outs: list of 8 dicts with 'out' (512, 512) -> (2, 512, 64, 32)."""
    per_batch = [
        np.concatenate([outs[b * 4 + t]["out"] for t in range(4)], axis=0)
        for b in range(B)
    ]
    return np.stack(per_batch).reshape(B, C, 64, 32).astype(np.float32)


def kernel(**inputs):
    from concourse.bass_utils import run_bass_kernel_spmd

    nc = get_nc()
    in_maps = make_in_maps(**inputs)
    res = run_bass_kernel_spmd(nc, in_maps, list(range(N_CORES)))
    return assemble(res.results)



# revision 15
# speedup vs baseline: 1.0506x; 1.0506x over previous
"""MultiHeadChannelAttention Bass kernel for 8 Trainium2 NeuronCores.

Problem (hardcoded shapes): x (2, 512, 64, 32) fp32; Wq/Wk/Wv/Wfc (512, 512);
biases (512,). Reference math per batch b, with X = x[b].reshape(2048, 512):
  Q = X Wq^T + bq ; K = X Wk^T + bk ; V = X Wv^T + bv   (heads of 64 dims)
  out = softmax(QK^T/8) V  (per head), concat heads, @ Wfc^T + bfc

Sharding: 8 cores = 2 batches x 4 token-blocks of 512 tokens. Each core
computes K/V for all 2048 tokens of its batch (4x redundant), Q/attention/fc
only for its 512-token block. No cross-core communication; the host only
slices inputs and concatenates outputs. Tokens are rotated per-core so the
core's own block sits at columns 0:512 of X^T — the Q projection then reads
the same xt tiles as K/V and no separate xq tensor is shipped.

Device layouts (all matmul-friendly, weights pre-transposed on host):
  XT  [512c, 2048t]  = X^T (rotated) in two column-half tiles per chunk
  KT [512, 2048] = (Wk X^T + bk);  QT [512, 512]
  scoresT [j, i] per head via row-tiled K=64 matmul pairs (2 heads/PE pass,
  concurrent in the PE array via distinct row groups)
  exp on ScalarE from 2-bank PSUM; on ~1/4 of tiles (always the last js
  of each pair, so boundary attnVs never wait on ScalarE) a DVE
  Schraudolph fast-exp (bf16-bit-pattern trick, one TENSOR_SCALAR) stands in
  so the exp stream isn't ScalarE-paced; attnV with ones-column (M=65) so the
  softmax denominator falls out of the same matmul; fc consumes attnout^T
  directly. bv is folded into the fc bias on host (softmax rows sum to 1).
  Softmax reciprocal = exp(-ln(x)) on ScalarE, batched [2, 512] per pair,
  broadcast to both head rows with a single fp16 selector matmul (fp32
  matmuls cost two half-speed PE passes). Output leaves as bf16; the host
  casts back to fp32.
"""

import numpy as np
import ml_dtypes

N_CORES = 8
B, C, N_TOK, TB = 2, 512, 2048, 512
HEADS, DK = 8, 64
NCH = C // 128  # channel chunks (4)
NJT = N_TOK // 128  # key-token tiles (16)
NTT = TB // 128  # fc token tiles (4)
HT = N_TOK // 2  # xt column-half (1024)

# Schraudolph fast-exp constants for bf16 bit patterns, including the 0.125
# attention scale: i16 = round(score * (0.125*128/ln2) + (127*128 - 5.5))
FEXP_C1 = 0.125 * 128.0 / float(np.log(2.0))
FEXP_C2 = 127.0 * 128.0 - 5.25
# iterations (per head-pair) whose exp runs on DVE instead of ScalarE.
# j=13/15 are always DVE so the pair's last attnVs never wait on the
# ScalarE exp queue at the boundary funnel.
FEXP_J = {
    0: (13, 15),
    1: (1, 7, 13, 15),
    2: (1, 7, 13, 15),
    3: (1, 3, 5, 9, 11, 13, 15),
}

_CACHE = {}


def _install_tile_drain_patch():
    """The end-of-kernel Tile drain can carry several sem waits; this
    walrus build allows one wait per non-EVSEM instruction. Split the
    waits across a chain of drains."""
    import bass_rust
    from concourse import tile as _tile
    from concourse.vector_clock import ScopedClock

    if getattr(_tile.TileContext, "_drain_patch_installed", False):
        return

    def _patched(self, tick_clock, wait_clock):
        nc = self.nc
        drain_inst = nc.sync.drain()
        wait_clock.add_sem_waits(
            drain_inst.ins, ScopedClock({None: tick_clock.global_clock})
        )
        si = drain_inst.ins.sync_info
        if si is not None and len(si.on_wait) > 1:
            waits = list(si.on_wait)
            drain_inst.ins.sync_info = bass_rust.SyncInfo(
                on_wait=[waits[0]], on_update=list(si.on_update)
            )
            for w in waits[1:]:
                extra = nc.sync.drain()
                extra.ins.sync_info = bass_rust.SyncInfo(on_wait=[w], on_update=[])
        nc.all_engine_barrier()
        assert self.sems is not None
        popped = nc._tile_sem_poison_stack.pop()
        assert popped is self._sem_poison
        nc.clear_and_free_semaphores(list(self.sems.allocated().values()))
        nc.all_engine_barrier()

    _tile.TileContext._drain_and_barrier = _patched
    _tile.TileContext._drain_patch_installed = True


def _split_multi_waits(nc):
    """This walrus build accepts one sync wait per instruction (two on
    EVSEM). Tile can attach two; move extras onto preceding NOPs."""
    import concourse.mybir as mybir

    for f in nc.m.functions:
        for bb in f.blocks:
            out = []
            changed = False
            for ins in bb.instructions:
                si = ins.sync_info
                limit = 2 if isinstance(ins, mybir.InstEventSemaphore) else 1
                if si is not None and len(si.on_wait) > limit:
                    waits = list(si.on_wait)
                    keep = waits[-limit:]
                    for i, w in enumerate(waits[:-limit]):
                        nop = mybir.InstNoOp(
                            name=f"{ins.name}_w{i}",
                            engine=ins.engine,
                            sync_info=mybir.SyncInfo(on_wait=[w], on_update=[]),
                            bass_nofuse=True,
                        )
                        nc.register_instruction(nop, overwrite=True)
                        out.append(nop)
                    ins.sync_info = mybir.SyncInfo(
                        on_wait=keep, on_update=list(si.on_update)
                    )
                    changed = True
                out.append(ins)
            if changed:
                bb.instructions = out


def _build():
    import concourse.bass as bass
    import concourse.mybir as mybir
    import concourse.tile as tile
    from concourse.bass import ts

    dt = mybir.dt
    f32, bf16, i16, f16 = dt.float32, dt.bfloat16, dt.int16, dt.float16
    Exp = mybir.ActivationFunctionType.Exp
    Ln = mybir.ActivationFunctionType.Ln
    Mult, Add = mybir.AluOpType.mult, mybir.AluOpType.add

    nc = bass.Bass()
    # weights are host-interleaved to [128, NCH*cols] so each DMA moves
    # one big per-partition span (large DMA packets) while chunk c still
    # slices out as [:, c*cols : ...] with partition p = channel 128c+p
    xt_d = nc.dram_tensor("xt", [C, N_TOK], bf16, kind="ExternalInput")
    wqT_d = nc.dram_tensor("wqT", [128, NCH * C], bf16, kind="ExternalInput")
    wkT_d = nc.dram_tensor("wkT", [128, NCH * C], bf16, kind="ExternalInput")
    wvT_d = nc.dram_tensor("wvT", [128, NCH * C], bf16, kind="ExternalInput")
    wfT_d = nc.dram_tensor("wfT", [128, NCH * C], bf16, kind="ExternalInput")
    bias_d = nc.dram_tensor("bias", [128, 2 * NCH], f32, kind="ExternalInput")
    bfc_d = nc.dram_tensor("bfc", [1, C], bf16, kind="ExternalInput")
    out_d = nc.dram_tensor("out", [TB, C], bf16, kind="ExternalOutput")

    with tile.TileContext(nc) as tc:
        with (
            tc.tile_pool(name="wp", bufs=1) as wp,
            tc.tile_pool(name="data", bufs=1) as data,
            tc.tile_pool(name="ep", bufs=6) as ep,
            tc.tile_pool(name="np_", bufs=2) as npool,
            tc.tile_pool(name="scp", bufs=2, space=bass.MemorySpace.PSUM) as scp,
            tc.tile_pool(name="ap_", bufs=1, space=bass.MemorySpace.PSUM) as apool,
            tc.tile_pool(name="aux", bufs=2, space=bass.MemorySpace.PSUM) as aux,
        ):
            # ---- constants / weights (merged [128, NCH*cols] tiles) ----
            wq_all = wp.tile([128, NCH * C], bf16, tag="wq", name="wq_all")
            wk_all = wp.tile([128, NCH * C], bf16, tag="wk", name="wk_all")
            wv_all = wp.tile([128, NCH * C], bf16, tag="wv", name="wv_all")
            wf_all = wp.tile([128, NCH * C], bf16, tag="wf", name="wf_all")
            wq = [wq_all[:, ts(c, C)] for c in range(NCH)]
            wk = [wk_all[:, ts(c, C)] for c in range(NCH)]
            wv = [wv_all[:, ts(c, C)] for c in range(NCH)]
            wf = [wf_all[:, ts(c, C)] for c in range(NCH)]
            bias_all = wp.tile([128, 2 * NCH], f32, tag="bias", name="bias_all")
            bqt = [bias_all[:, d : d + 1] for d in range(NCH)]
            bkt = [bias_all[:, NCH + d : NCH + d + 1] for d in range(NCH)]
            bfct = wp.tile([1, C], bf16, tag="bfct", name="bfct")
            ones_t = wp.tile([128, TB], bf16, tag="ones", name="ones_t")
            nc.gpsimd.memset(ones_t[:], 1.0)
            ones_f = wp.tile([128, 64], f32, tag="onesf", name="ones_f")
            nc.vector.memset(ones_f[:], 1.0)
            # selector for the denominator broadcast: one fp16 matmul maps
            # rcp row 0 -> out partitions 0-63 and row 32 -> 64-127
            sel = wp.tile([33, 128], f16, tag="sel", name="sel")
            nc.vector.memset(sel[:], 0.0)
            nc.vector.memset(sel[0:1, 0:64], 1.0)
            nc.vector.memset(sel[32:33, 64:128], 1.0)

            # ---- activations in: two column-half tiles per channel chunk ----
            xta = [
                data.tile([128, HT], bf16, tag=f"xta{c}", name=f"xta{c}")
                for c in range(NCH)
            ]
            xtb = [
                data.tile([128, HT], bf16, tag=f"xtb{c}", name=f"xtb{c}")
                for c in range(NCH)
            ]

            def xk(jb):  # xt tile + column block for K-proj token block jb
                return (xta if jb < 2 else xtb), jb % 2

            def xv(j):  # xt tile + column tile for V-proj token tile j
                return (xta if j < 8 else xtb), j % 8

            # ---- input DMAs over the three issue paths (SP/ACT HWDGE +
            # gpsimd SWDGE). Weights lead on scalar+gpsimd; the xt first
            # halves (needed by Q proj and the early K/V token blocks)
            # lead on sync ----
            nc.scalar.dma_start(out=wq_all[:], in_=wqT_d[:])
            nc.gpsimd.dma_start(out=wk_all[:], in_=wkT_d[:])
            nc.sync.dma_start(out=xta[0][:], in_=xt_d[ts(0, 128), 0:HT])
            nc.sync.dma_start(out=xta[1][:], in_=xt_d[ts(1, 128), 0:HT])
            nc.sync.dma_start(out=xta[2][:], in_=xt_d[ts(2, 128), 0:HT])
            nc.sync.dma_start(out=xta[3][:], in_=xt_d[ts(3, 128), 0:HT])
            nc.scalar.dma_start(out=wv_all[:], in_=wvT_d[:])
            nc.gpsimd.dma_start(out=xtb[0][:], in_=xt_d[ts(0, 128), HT:N_TOK])
            nc.scalar.dma_start(out=xtb[1][:], in_=xt_d[ts(1, 128), HT:N_TOK])
            nc.gpsimd.dma_start(out=xtb[2][:], in_=xt_d[ts(2, 128), HT:N_TOK])
            nc.scalar.dma_start(out=xtb[3][:], in_=xt_d[ts(3, 128), HT:N_TOK])
            nc.sync.dma_start(out=bias_all[:], in_=bias_d[:])
            nc.gpsimd.dma_start(out=wf_all[:], in_=wfT_d[:])
            nc.sync.dma_start(out=bfct[:], in_=bfc_d[:])

            # trigger the natural_log_exp ACT table load during the DMA
            # window instead of right before the first real exp
            tbl = npool.tile([1, 64], f32, tag="tbl", bufs=1, name="tbl")
            nc.scalar.activation(out=tbl[:], in_=ones_f[0:1, :], func=Ln)

            # PE warmup: one dummy accumulation chain on the ones tile keeps
            # the HAM activity monitor busy through the input-load window so
            # the first real projections run at 2.4 GHz
            warm = aux.tile([128, TB], f32, tag="aux", name="warm")
            for r in range(10):
                nc.tensor.matmul(
                    warm[:], ones_t[0:1, 0:128], ones_t[0:1, :],
                    start=(r == 0), stop=(r == 9),
                )

            # ---- persistent intermediates ----
            kt = [data.tile([128, N_TOK], bf16, tag=f"kt{d}", name=f"kt{d}") for d in range(NCH)]
            qt = [data.tile([128, TB], bf16, tag=f"qt{d}", name=f"qt{d}") for d in range(NCH)]
            vpad = [
                data.tile([128, HEADS, DK + 1], bf16, tag=f"vp{j}", name=f"vp{j}")
                for j in range(NJT)
            ]
            att = [
                data.tile([128, TB], bf16, tag=f"att{c}", name=f"att{c}")
                for c in range(NCH)
            ]

            def proj_q(d):
                """Q^T d-tile (128 chans = heads 2d, 2d+1) + bias."""
                qp = aux.tile([128, TB], f32, tag="aux", name=f"qp{d}")
                for c in range(NCH):
                    nc.tensor.matmul(
                        qp[:], wq[c][:, ts(d, 128)], xta[c][:, 0:TB],
                        start=(c == 0), stop=(c == NCH - 1),
                    )
                nc.vector.tensor_scalar_add(out=qt[d][:], in0=qp[:], scalar1=bqt[d][:])

            def proj_k(d, jb):
                """K^T d-tile, token block jb + bias."""
                xt_half, hb = xk(jb)
                kp = aux.tile([128, TB], f32, tag="aux", name=f"kp{d}_{jb}")
                for c in range(NCH):
                    nc.tensor.matmul(
                        kp[:], wk[c][:, ts(d, 128)], xt_half[c][:, ts(hb, TB)],
                        start=(c == 0), stop=(c == NCH - 1),
                    )
                nc.vector.tensor_scalar_add(
                    out=kt[d][:, ts(jb, TB)], in0=kp[:], scalar1=bkt[d][:]
                )

            def proj_kq(d):
                proj_q(d)
                for jb in range(N_TOK // TB):
                    proj_k(d, jb)

            def proj_v(j):
                """V j-tile -> padded [128, 8, 65] with ones in column 64.
                The PSUM->SBUF evacuations alternate between ScalarE and
                DVE so neither engine paces pair 0 (GpSimd cannot read
                PSUM, so it can't take these)."""
                xt_half, hj = xv(j)
                vp = aux.tile([128, C], f32, tag="aux", name=f"vpp{j}")
                for c in range(NCH):
                    nc.tensor.matmul(
                        vp[:], xt_half[c][:, ts(hj, 128)], wv[c][:],
                        start=(c == 0), stop=(c == NCH - 1),
                    )
                src = vp[:].rearrange("p (h d) -> p h d", h=HEADS)
                if j % 2 == 0:
                    nc.scalar.copy(out=vpad[j][:, :, 0:DK], in_=src)
                else:
                    nc.vector.tensor_copy(out=vpad[j][:, :, 0:DK], in_=src)
                nc.vector.memset(vpad[j][:, :, DK : DK + 1], 1.0)

            # ---- main pipeline ----
            def norm_gather(pp, a0, a1):
                """Collect the pair's two softmax denominators into one
                tile (rows 0 and 32 — matmul rhs base partitions must be
                0/32/64) so the ln/exp reciprocal runs as one ScalarE call
                per function instead of two. Rows 1-31 carry garbage that
                nothing reads. Reads row 64 straight from the PSUM
                accumulators so it doesn't wait on the SBUF evacuation.
                Rows 1-31 are memset to 1.0: the selector matmul contracts
                over all 33 rcp rows, and ln/exp of stale SBUF garbage can
                be inf/nan, which survives a 0-weight (0*nan=nan)."""
                dn = npool.tile([33, TB], f32, tag="dn", bufs=2, name=f"dn{pp}")
                nc.vector.memset(dn[0:32, :], 1.0)
                nc.vector.tensor_copy(out=dn[0:1, :], in_=a0[64:65, :])
                nc.vector.tensor_copy(out=dn[32:33, :], in_=a1[64:65, :])
                return dn

            def norm_recip(pp, dn):
                """Batched reciprocal of both denominators on ScalarE as
                exp(-ln(x)) — both functions live in one ACT table set, and
                it keeps the slow iterative divide off DVE. fp16 output so
                the broadcast matmul runs at 1 cycle/row (an fp32 rhs costs
                two half-speed PE passes)."""
                lnt = npool.tile([33, TB], f32, tag="lnt", bufs=2, name=f"lnt{pp}")
                nc.scalar.activation(out=lnt[:], in_=dn[:], func=Ln)
                rcp = npool.tile([33, TB], f16, tag="rcp", bufs=2, name=f"rcp{pp}")
                nc.scalar.activation(out=rcp[:], in_=lnt[:], func=Exp, scale=-1.0)
                return rcp

            def norm_apply(pp, asb, rcp):
                """Both heads at once: one selector matmul broadcasts rcp
                rows 0/32 onto out partitions 0-63/64-127, one DVE multiply
                normalizes the whole pair."""
                rb = aux.tile([128, TB], f32, tag="aux", name=f"rb{pp}")
                nc.tensor.matmul(rb[:], sel[:, :], rcp[:, :])
                nc.vector.tensor_mul(out=att[pp][:, :], in0=asb[:, :], in1=rb[:])

            proj_q(0)
            proj_k(0, 0)
            fps = []  # fc PSUM accumulators; t=0/1 filled in pair 3's loop
            prev = None  # previous pair's SBUF accumulator copies
            prev_dn = None
            prev_rcp = None
            for p in range(NCH):  # head pair p = heads 2p, 2p+1
                a0 = apool.tile([DK + 1, TB], f32, tag="a0", name=f"a0_{p}")
                a1 = apool.tile([DK + 1, TB], f32, tag="a1", name=f"a1_{p}")
                fexp_js = FEXP_J[p]

                def attn_v(j, e):
                    nc.tensor.matmul(
                        a0[:], vpad[j][:, 2 * p, :], e[:, 0:TB],
                        start=(j == 0), stop=(j == NJT - 1),
                    )
                    nc.tensor.matmul(
                        a1[:], vpad[j][:, 2 * p + 1, :], e[:, TB : 2 * TB],
                        start=(j == 0), stop=(j == NJT - 1),
                    )

                pend = None  # (j, e) whose attnV is deferred one iteration
                for j in range(NJT):
                    # pair 0: the rest of K^T, paced with the xt DMA stream
                    # (jb 2/3 need the xt second halves, which land late)
                    if p == 0 and j in (1, 4, 6):
                        proj_k(0, {1: 1, 4: 2, 6: 3}[j])
                    # next pair's K/Q projection: the early pieces (needed
                    # by its first scores) run mid-pair; the late jb pieces
                    # are emitted at the boundary below as PE filler.
                    # Pair 0 already carries the V projection, so all of
                    # pair 1's pieces move to the boundary instead.
                    if 0 < p < NCH - 1:
                        if j == 10:
                            proj_q(p + 1)
                        elif j in (12, 14):
                            proj_k(p + 1, (j - 12) // 2)
                    # previous pair's normalization, deferred into this
                    # pair's loop so its reciprocal/broadcast work doesn't
                    # gate PE at the boundary. The last pair normalizes
                    # early (j=0-2) so att[2] is ready for in-loop fc
                    # prefill — real PE work that keeps the clock monitor
                    # from throttling the otherwise projection-free pair 3.
                    if prev is not None:
                        if p < NCH - 1:
                            if j == 3:
                                prev_rcp = norm_recip(p - 1, prev_dn)
                            elif j == 5:
                                norm_apply(p - 1, prev, prev_rcp)
                        else:
                            if j == 0:
                                prev_rcp = norm_recip(p - 1, prev_dn)
                            elif j == 1:
                                norm_apply(p - 1, prev, prev_rcp)
                    # fc prefill for token chunks 0/1 interleaved into the
                    # last pair's loop (aux PSUM slots are free here). These
                    # full-array matmuls also re-warm the clock monitor,
                    # which the half-array sc/attnV mix cannot.
                    if p == NCH - 1 and j in (3, 5, 7, 9, 11, 13):
                        t = int(j >= 9)
                        c = ((j - 3) % 6) // 2
                        if c == 0:
                            fp = aux.tile([128, C], f32, tag="aux", name=f"fp{t}")
                            fps.append(fp)
                            nc.tensor.matmul(
                                fp[:], ones_t[0:1, 0:128], bfct[:],
                                start=True, stop=False,
                            )
                        nc.tensor.matmul(
                            fps[t][:], att[c][:, ts(t, 128)], wf[c][:],
                            start=False, stop=False,
                        )
                    sc = scp.tile([128, 2 * TB], f32, tag="sc", name=f"sc{p}_{j}")
                    nc.tensor.matmul(
                        sc[:, 0:TB], kt[p][0:64, ts(j, 128)], qt[p][0:64, :]
                    )
                    nc.tensor.matmul(
                        sc[:, TB : 2 * TB], kt[p][64:128, ts(j, 128)], qt[p][64:128, :]
                    )
                    e = ep.tile([128, 2 * TB], bf16, tag="e", name=f"e{p}_{j}")
                    if j in fexp_js:
                        # Schraudolph fast-exp on DVE: bf16 bit pattern via
                        # int16 round(x*C1 + C2); error ~3% per weight,
                        # washes out over the 2048-key softmax average
                        nc.vector.tensor_scalar(
                            out=e[:].bitcast(i16), in0=sc[:],
                            scalar1=FEXP_C1, scalar2=FEXP_C2, op0=Mult, op1=Add,
                        )
                    else:
                        nc.scalar.activation(out=e[:], in_=sc[:], func=Exp, scale=0.125)
                    # V projection emitted after scores/exp so a late wv/xt
                    # DMA can't block the PE stream ahead of the scores
                    if p == 0:
                        proj_v(j)
                    # attnV deferred one iteration: the tensor queue is
                    # strict FIFO, so attnV(j) right here would block the
                    # already-queued sc(j+1) behind exp(j). One iteration
                    # of skew keeps PE from ever waiting on the exp stream.
                    if pend is not None:
                        attn_v(*pend)
                    pend = (j, e)
                attn_v(*pend)
                # evacuate both accumulators into one [128, TB] SBUF tile
                # (head 2p rows 0-63, head 2p+1 rows 64-127) — frees the
                # PSUM banks and feeds the single-multiply normalization
                asb = npool.tile([128, TB], f32, tag="asb", bufs=2, name=f"asb_{p}")
                nc.vector.tensor_copy(out=asb[0:64, :], in_=a0[0:64, :])
                nc.scalar.copy(out=asb[64:128, :], in_=a1[0:64, :])
                prev = asb
                prev_dn = norm_gather(p, a0, a1)
                # boundary PE filler: the next pair's late K pieces (not
                # needed until its scores j>=8) keep HAM warm through the
                # cross-pair dependency funnel
                if p == 0:
                    proj_q(1)
                    proj_k(1, 0)
                    proj_k(1, 1)
                if p + 1 < NCH:
                    proj_k(p + 1, 2)
                    proj_k(p + 1, 3)

            # ---- tail: all four fc tiles pre-accumulate bias + the first
            # three chunks (PE filler while the last pair's reciprocals run
            # on ScalarE); only the final chunk waits on normalize(3) ----
            def fc_prefill(t, fp):
                nc.tensor.matmul(
                    fp[:], ones_t[0:1, 0:128], bfct[:], start=True, stop=False
                )
                for c in range(NCH - 1):
                    nc.tensor.matmul(
                        fp[:], att[c][:, ts(t, 128)], wf[c][:],
                        start=False, stop=False,
                    )

            for t in (2, 3):
                # scores pool is draining by now; reuse its slots
                fp = scp.tile([128, C], f32, tag="sc", name=f"fp{t}")
                fc_prefill(t, fp)
                fps.append(fp)
            # last pair's normalization, pipelined per fc token chunk: the
            # reciprocal is split into [33, 128] pieces so the first fc
            # final starts ~1us earlier and the whole chunk chain
            # (recip -> broadcast -> multiply -> fc -> evac -> DMA)
            # overlaps across chunks. rb3 lives in the attnV accumulator
            # pool (free after the acopies); each chunk's broadcast writes
            # its own column window of the one PSUM bank.
            rb3 = apool.tile([128, TB], f32, tag="a0", name="rb3")
            out_q = [nc.sync, nc.scalar, nc.gpsimd, nc.sync]
            for t in range(NTT):
                lnt_t = npool.tile([33, 128], f32, tag="lnt", bufs=2, name=f"lnt3_{t}")
                nc.scalar.activation(out=lnt_t[:], in_=prev_dn[:, ts(t, 128)], func=Ln)
                rcp_t = npool.tile([33, 128], f16, tag="rcp", bufs=2, name=f"rcp3_{t}")
                nc.scalar.activation(out=rcp_t[:], in_=lnt_t[:], func=Exp, scale=-1.0)
                nc.tensor.matmul(rb3[:, ts(t, 128)], sel[:, :], rcp_t[:, :])
                nc.vector.tensor_mul(
                    out=att[NCH - 1][:, ts(t, 128)],
                    in0=prev[:, ts(t, 128)],
                    in1=rb3[:, ts(t, 128)],
                )
                nc.tensor.matmul(
                    fps[t][:], att[NCH - 1][:, ts(t, 128)], wf[NCH - 1][:],
                    start=False, stop=True,
                )
                ot = npool.tile([128, C], bf16, tag="ot", bufs=4, name=f"ot{t}")
                if t % 2 == 0:
                    nc.vector.tensor_copy(out=ot[:], in_=fps[t][:])
                else:
                    nc.scalar.copy(out=ot[:], in_=fps[t][:])
                out_q[t].dma_start(out=out_d[ts(t, 128), :], in_=ot[:])

    _split_multi_waits(nc)
    nc.finalize()
    return nc


def get_nc():
    if "nc" not in _CACHE:
        _install_tile_drain_patch()
        _CACHE["nc"] = _build()
    return _CACHE["nc"]


def make_in_maps(x, Wq, bq, Wk, bk, Wv, bv, Wfc, bfc):
    bf = ml_dtypes.bfloat16
    x = np.asarray(x, np.float32)
    Wq, Wk, Wv, Wfc = (np.asarray(w, np.float32) for w in (Wq, Wk, Wv, Wfc))
    bq, bk, bv, bfc = (np.asarray(v, np.float32) for v in (bq, bk, bv, bfc))

    def interleave(wT):
        # [C, cols] -> [128, NCH*cols] with chunk c at columns [c*cols:...]
        cols = wT.shape[1]
        return np.ascontiguousarray(
            wT.reshape(NCH, 128, cols).transpose(1, 0, 2).reshape(128, NCH * cols)
        )

    bfc_folded = (Wfc @ bv + bfc).reshape(1, C).astype(bf)
    wqT = interleave(np.ascontiguousarray(Wq.T).astype(bf))
    wkT = interleave(np.ascontiguousarray(Wk.T).astype(bf))
    wvT = interleave(np.ascontiguousarray(Wv.T).astype(bf))
    wfT = interleave(np.ascontiguousarray(Wfc.T).astype(bf))
    bias_c = np.concatenate(
        [bq.reshape(NCH, 128).T, bk.reshape(NCH, 128).T], axis=1
    ).astype(np.float32)

    in_maps = []
    for core in range(N_CORES):
        b, t = divmod(core, N_TOK // TB)
        XT = np.ascontiguousarray(x[b].reshape(N_TOK, C).T).astype(bf)
        # rotate tokens so this core's block sits at columns 0:TB — the Q
        # projection then reads xt directly (attention is j-permutation
        # invariant, so K/V token order doesn't matter as long as kt and
        # vpad agree, which they do: both derive from this xt)
        perm = np.r_[t * TB : (t + 1) * TB, 0 : t * TB, (t + 1) * TB : N_TOK]
        in_maps.append(
            {
                "xt": np.ascontiguousarray(XT[:, perm]),
                "wqT": wqT,
                "wkT": wkT,
                "wvT": wvT,
                "wfT": wfT,
                "bias": bias_c,
                "bfc": bfc_folded,
            }
        )
    return in_maps


def assemble(outs):
    """outs: list of 8 dicts with 'out' (512, 512) -> (2, 512, 64, 32)."""
    per_batch = [
        np.concatenate([outs[b * 4 + t]["out"] for t in range(4)], axis=0)
        for b in range(B)
    ]
    return np.stack(per_batch).reshape(B, C, 64, 32).astype(np.float32)


def kernel(**inputs):
    from concourse.bass_utils import run_bass_kernel_spmd

    nc = get_nc()
    in_maps = make_in_maps(**inputs)
    res = run_bass_kernel_spmd(nc, in_maps, list(range(N_CORES)))
    return assemble(res.results)



# revision 20
# speedup vs baseline: 1.0738x; 1.0221x over previous
"""MultiHeadChannelAttention Bass kernel for 8 Trainium2 NeuronCores.

Problem (hardcoded shapes): x (2, 512, 64, 32) fp32; Wq/Wk/Wv/Wfc (512, 512);
biases (512,). Reference math per batch b, with X = x[b].reshape(2048, 512):
  Q = X Wq^T + bq ; K = X Wk^T + bk ; V = X Wv^T + bv   (heads of 64 dims)
  out = softmax(QK^T/8) V  (per head), concat heads, @ Wfc^T + bfc

Sharding: 8 cores = 2 batches x 4 token-blocks of 512 tokens. Each core
computes K/V for all 2048 tokens of its batch (4x redundant), Q/attention/fc
only for its 512-token block. No cross-core communication; the host only
slices inputs and concatenates outputs. Tokens are rotated per-core so the
core's own block sits at columns 0:512 of X^T — the Q projection then reads
the same xt tiles as K/V and no separate xq tensor is shipped.

Device layouts (all matmul-friendly, weights pre-transposed on host):
  XT  [512c, 2048t]  = X^T (rotated) in two column-half tiles per chunk
  KT [512, 2048] = (Wk X^T + bk);  QT [512, 512]
  scoresT [j, i] per head via row-tiled K=64 matmul pairs (2 heads/PE pass,
  concurrent in the PE array via distinct row groups)
  exp on ScalarE from 2-bank PSUM; on ~1/4 of tiles (always the last js
  of each pair, so boundary attnVs never wait on ScalarE) a DVE
  Schraudolph fast-exp (bf16-bit-pattern trick, one TENSOR_SCALAR) stands in
  so the exp stream isn't ScalarE-paced; attnV with ones-column (M=65) so the
  softmax denominator falls out of the same matmul; fc consumes attnout^T
  directly. bv is folded into the fc bias on host (softmax rows sum to 1).
  Softmax reciprocal = exp(-ln(x)) on ScalarE, batched [2, 512] per pair,
  broadcast to both head rows with a single fp16 selector matmul (fp32
  matmuls cost two half-speed PE passes). Output leaves as bf16; the host
  casts back to fp32.
"""

import numpy as np
import ml_dtypes

N_CORES = 8
B, C, N_TOK, TB = 2, 512, 2048, 512
HEADS, DK = 8, 64
NCH = C // 128  # channel chunks (4)
NJT = N_TOK // 128  # key-token tiles (16)
NTT = TB // 128  # fc token tiles (4)
HT = N_TOK // 2  # xt column-half (1024)

# Schraudolph fast-exp constants for bf16 bit patterns, including the 0.125
# attention scale: i16 = round(score * (0.125*128/ln2) + (127*128 - 5.5))
FEXP_C1 = 0.125 * 128.0 / float(np.log(2.0))
FEXP_C2 = 127.0 * 128.0 - 5.25
# iterations (per head-pair) whose exp runs on DVE instead of ScalarE.
# j=13/15 are always DVE so the pair's last attnVs never wait on the
# ScalarE exp queue at the boundary funnel.
FEXP_J = {
    0: (13, 15),
    1: (1, 7, 13, 15),
    2: (1, 7, 13, 15),
    3: (1, 3, 5, 9, 11, 13, 15),
}

_CACHE = {}


def _install_tile_drain_patch():
    """The end-of-kernel Tile drain can carry several sem waits; this
    walrus build allows one wait per non-EVSEM instruction. Split the
    waits across a chain of drains."""
    import bass_rust
    from concourse import tile as _tile
    from concourse.vector_clock import ScopedClock

    if getattr(_tile.TileContext, "_drain_patch_installed", False):
        return

    def _patched(self, tick_clock, wait_clock):
        nc = self.nc
        drain_inst = nc.sync.drain()
        wait_clock.add_sem_waits(
            drain_inst.ins, ScopedClock({None: tick_clock.global_clock})
        )
        si = drain_inst.ins.sync_info
        if si is not None and len(si.on_wait) > 1:
            waits = list(si.on_wait)
            drain_inst.ins.sync_info = bass_rust.SyncInfo(
                on_wait=[waits[0]], on_update=list(si.on_update)
            )
            for w in waits[1:]:
                extra = nc.sync.drain()
                extra.ins.sync_info = bass_rust.SyncInfo(on_wait=[w], on_update=[])
        nc.all_engine_barrier()
        assert self.sems is not None
        popped = nc._tile_sem_poison_stack.pop()
        assert popped is self._sem_poison
        nc.clear_and_free_semaphores(list(self.sems.allocated().values()))
        nc.all_engine_barrier()

    _tile.TileContext._drain_and_barrier = _patched
    _tile.TileContext._drain_patch_installed = True


def _split_multi_waits(nc):
    """This walrus build accepts one sync wait per instruction (two on
    EVSEM). Tile can attach two; move extras onto preceding NOPs."""
    import concourse.mybir as mybir

    for f in nc.m.functions:
        for bb in f.blocks:
            out = []
            changed = False
            for ins in bb.instructions:
                si = ins.sync_info
                limit = 2 if isinstance(ins, mybir.InstEventSemaphore) else 1
                if si is not None and len(si.on_wait) > limit:
                    waits = list(si.on_wait)
                    keep = waits[-limit:]
                    for i, w in enumerate(waits[:-limit]):
                        nop = mybir.InstNoOp(
                            name=f"{ins.name}_w{i}",
                            engine=ins.engine,
                            sync_info=mybir.SyncInfo(on_wait=[w], on_update=[]),
                            bass_nofuse=True,
                        )
                        nc.register_instruction(nop, overwrite=True)
                        out.append(nop)
                    ins.sync_info = mybir.SyncInfo(
                        on_wait=keep, on_update=list(si.on_update)
                    )
                    changed = True
                out.append(ins)
            if changed:
                bb.instructions = out


def _build():
    import concourse.bass as bass
    import concourse.mybir as mybir
    import concourse.tile as tile
    from concourse.bass import ts

    dt = mybir.dt
    f32, bf16, i16, f16 = dt.float32, dt.bfloat16, dt.int16, dt.float16
    Exp = mybir.ActivationFunctionType.Exp
    Ln = mybir.ActivationFunctionType.Ln
    Mult, Add = mybir.AluOpType.mult, mybir.AluOpType.add

    nc = bass.Bass()
    # weights are host-interleaved to [128, NCH*cols] so each DMA moves
    # one big per-partition span (large DMA packets) while chunk c still
    # slices out as [:, c*cols : ...] with partition p = channel 128c+p
    xt_d = nc.dram_tensor("xt", [C, N_TOK], bf16, kind="ExternalInput")
    wqT_d = nc.dram_tensor("wqT", [128, NCH * C], bf16, kind="ExternalInput")
    wkT_d = nc.dram_tensor("wkT", [128, NCH * C], bf16, kind="ExternalInput")
    wvT_d = nc.dram_tensor("wvT", [128, NCH * C], bf16, kind="ExternalInput")
    wfT_d = nc.dram_tensor("wfT", [128, NCH * C], bf16, kind="ExternalInput")
    bias_d = nc.dram_tensor("bias", [128, 2 * NCH], f32, kind="ExternalInput")
    bfc_d = nc.dram_tensor("bfc", [1, C], bf16, kind="ExternalInput")
    out_d = nc.dram_tensor("out", [TB, C], bf16, kind="ExternalOutput")

    with tile.TileContext(nc) as tc:
        with (
            tc.tile_pool(name="wp", bufs=1) as wp,
            tc.tile_pool(name="data", bufs=1) as data,
            tc.tile_pool(name="ep", bufs=6) as ep,
            tc.tile_pool(name="np_", bufs=2) as npool,
            tc.tile_pool(name="scp", bufs=2, space=bass.MemorySpace.PSUM) as scp,
            tc.tile_pool(name="ap_", bufs=1, space=bass.MemorySpace.PSUM) as apool,
            tc.tile_pool(name="aux", bufs=2, space=bass.MemorySpace.PSUM) as aux,
        ):
            # ---- constants / weights (merged [128, NCH*cols] tiles).
            # wq/wk are d-major ((d, c) 128-col blocks) so the d=0 pieces
            # that gate the first scores ship as small early DMAs; wv/wf
            # stay c-major (consumed whole-chunk) ----
            wq_all = wp.tile([128, NCH * C], bf16, tag="wq", name="wq_all")
            wk_all = wp.tile([128, NCH * C], bf16, tag="wk", name="wk_all")
            wv_all = wp.tile([128, NCH * C], bf16, tag="wv", name="wv_all")
            wf_all = wp.tile([128, NCH * C], bf16, tag="wf", name="wf_all")

            def wqd(d, c):
                return wq_all[:, ts(d * NCH + c, 128)]

            def wkd(d, c):
                return wk_all[:, ts(d * NCH + c, 128)]

            wv = [wv_all[:, ts(c, C)] for c in range(NCH)]
            wf = [wf_all[:, ts(c, C)] for c in range(NCH)]
            bias_all = wp.tile([128, 2 * NCH], f32, tag="bias", name="bias_all")
            bqt = [bias_all[:, d : d + 1] for d in range(NCH)]
            bkt = [bias_all[:, NCH + d : NCH + d + 1] for d in range(NCH)]
            bfct = wp.tile([1, C], bf16, tag="bfct", name="bfct")
            ones_t = wp.tile([128, TB], bf16, tag="ones", name="ones_t")
            nc.gpsimd.memset(ones_t[:], 1.0)
            ones_f = wp.tile([128, 64], f32, tag="onesf", name="ones_f")
            nc.vector.memset(ones_f[:], 1.0)
            # selector for the denominator broadcast: one fp16 matmul maps
            # rcp row 0 -> out partitions 0-63 and row 32 -> 64-127
            sel = wp.tile([33, 128], f16, tag="sel", name="sel")
            nc.vector.memset(sel[:], 0.0)
            nc.vector.memset(sel[0:1, 0:64], 1.0)
            nc.vector.memset(sel[32:33, 64:128], 1.0)

            # ---- activations in: two column-half tiles per channel chunk ----
            xta = [
                data.tile([128, HT], bf16, tag=f"xta{c}", name=f"xta{c}")
                for c in range(NCH)
            ]
            xtb = [
                data.tile([128, HT], bf16, tag=f"xtb{c}", name=f"xtb{c}")
                for c in range(NCH)
            ]

            def xk(jb):  # xt tile + column block for K-proj token block jb
                return (xta if jb < 2 else xtb), jb % 2

            def xv(j):  # xt tile + column tile for V-proj token tile j
                return (xta if j < 8 else xtb), j % 8

            # ---- input DMAs over the three issue paths (SP/ACT HWDGE +
            # gpsimd SWDGE), each ring ~146 GB/s. Ordered by consumption
            # deadline: the d=0 weight blocks + xta gate the first scores
            # (~17us); wv gates the first attnV; xtb is needed from pair-0
            # j~6; wq/wk d1-3 from the pair-0 boundary; wf only at the fc
            # prefill (~100us) ----
            nc.sync.dma_start(out=bias_all[:], in_=bias_d[:])
            nc.scalar.dma_start(out=wq_all[:, 0 : NCH * 128], in_=wqT_d[:, 0 : NCH * 128])
            nc.scalar.dma_start(out=wk_all[:, 0 : NCH * 128], in_=wkT_d[:, 0 : NCH * 128])
            nc.sync.dma_start(out=xta[0][:], in_=xt_d[ts(0, 128), 0:HT])
            nc.gpsimd.dma_start(out=xta[2][:], in_=xt_d[ts(2, 128), 0:HT])
            nc.sync.dma_start(out=xta[1][:], in_=xt_d[ts(1, 128), 0:HT])
            nc.gpsimd.dma_start(out=xta[3][:], in_=xt_d[ts(3, 128), 0:HT])
            nc.scalar.dma_start(out=wv_all[:], in_=wvT_d[:])
            nc.scalar.dma_start(
                out=wq_all[:, NCH * 128 : NCH * C], in_=wqT_d[:, NCH * 128 : NCH * C]
            )
            nc.scalar.dma_start(
                out=wk_all[:, NCH * 128 : NCH * C], in_=wkT_d[:, NCH * 128 : NCH * C]
            )
            nc.sync.dma_start(out=xtb[0][:], in_=xt_d[ts(0, 128), HT:N_TOK])
            nc.gpsimd.dma_start(out=xtb[2][:], in_=xt_d[ts(2, 128), HT:N_TOK])
            nc.sync.dma_start(out=xtb[1][:], in_=xt_d[ts(1, 128), HT:N_TOK])
            nc.scalar.dma_start(out=xtb[3][:], in_=xt_d[ts(3, 128), HT:N_TOK])
            nc.gpsimd.dma_start(out=wf_all[:], in_=wfT_d[:])
            nc.sync.dma_start(out=bfct[:], in_=bfc_d[:])

            # trigger the natural_log_exp ACT table load during the DMA
            # window instead of right before the first real exp
            tbl = npool.tile([1, 64], f32, tag="tbl", bufs=1, name="tbl")
            nc.scalar.activation(out=tbl[:], in_=ones_f[0:1, :], func=Ln)

            # PE warmup: one dummy accumulation chain on the ones tile keeps
            # the HAM activity monitor busy through the input-load window so
            # the first real projections run at 2.4 GHz
            warm = aux.tile([128, TB], f32, tag="aux", name="warm")
            for r in range(10):
                nc.tensor.matmul(
                    warm[:], ones_t[0:1, 0:128], ones_t[0:1, :],
                    start=(r == 0), stop=(r == 9),
                )

            # ---- persistent intermediates ----
            kt = [data.tile([128, N_TOK], bf16, tag=f"kt{d}", name=f"kt{d}") for d in range(NCH)]
            qt = [data.tile([128, TB], bf16, tag=f"qt{d}", name=f"qt{d}") for d in range(NCH)]
            vpad = [
                data.tile([128, HEADS, DK + 1], bf16, tag=f"vp{j}", name=f"vp{j}")
                for j in range(NJT)
            ]
            att = [
                data.tile([128, TB], bf16, tag=f"att{c}", name=f"att{c}")
                for c in range(NCH)
            ]

            def proj_q(d):
                """Q^T d-tile (128 chans = heads 2d, 2d+1) + bias."""
                qp = aux.tile([128, TB], f32, tag="aux", name=f"qp{d}")
                for c in range(NCH):
                    nc.tensor.matmul(
                        qp[:], wqd(d, c), xta[c][:, 0:TB],
                        start=(c == 0), stop=(c == NCH - 1),
                    )
                nc.vector.tensor_scalar_add(out=qt[d][:], in0=qp[:], scalar1=bqt[d][:])

            def proj_k(d, jb):
                """K^T d-tile, token block jb + bias."""
                xt_half, hb = xk(jb)
                kp = aux.tile([128, TB], f32, tag="aux", name=f"kp{d}_{jb}")
                for c in range(NCH):
                    nc.tensor.matmul(
                        kp[:], wkd(d, c), xt_half[c][:, ts(hb, TB)],
                        start=(c == 0), stop=(c == NCH - 1),
                    )
                nc.vector.tensor_scalar_add(
                    out=kt[d][:, ts(jb, TB)], in0=kp[:], scalar1=bkt[d][:]
                )

            def proj_kq(d):
                proj_q(d)
                for jb in range(N_TOK // TB):
                    proj_k(d, jb)

            def proj_v(j):
                """V j-tile -> padded [128, 8, 65] with ones in column 64.
                The PSUM->SBUF evacuations alternate between ScalarE and
                DVE so neither engine paces pair 0 (GpSimd cannot read
                PSUM, so it can't take these)."""
                xt_half, hj = xv(j)
                vp = aux.tile([128, C], f32, tag="aux", name=f"vpp{j}")
                for c in range(NCH):
                    nc.tensor.matmul(
                        vp[:], xt_half[c][:, ts(hj, 128)], wv[c][:],
                        start=(c == 0), stop=(c == NCH - 1),
                    )
                src = vp[:].rearrange("p (h d) -> p h d", h=HEADS)
                if j % 2 == 0:
                    nc.scalar.copy(out=vpad[j][:, :, 0:DK], in_=src)
                else:
                    nc.vector.tensor_copy(out=vpad[j][:, :, 0:DK], in_=src)
                nc.vector.memset(vpad[j][:, :, DK : DK + 1], 1.0)

            # ---- main pipeline ----
            def norm_gather(pp, a0, a1):
                """Collect the pair's two softmax denominators into one
                tile (rows 0 and 32 — matmul rhs base partitions must be
                0/32/64) so the ln/exp reciprocal runs as one ScalarE call
                per function instead of two. Rows 1-31 carry garbage that
                nothing reads. Reads row 64 straight from the PSUM
                accumulators so it doesn't wait on the SBUF evacuation.
                Rows 1-31 are memset to 1.0: the selector matmul contracts
                over all 33 rcp rows, and ln/exp of stale SBUF garbage can
                be inf/nan, which survives a 0-weight (0*nan=nan)."""
                dn = npool.tile([33, TB], f32, tag="dn", bufs=2, name=f"dn{pp}")
                nc.vector.memset(dn[0:32, :], 1.0)
                nc.vector.tensor_copy(out=dn[0:1, :], in_=a0[64:65, :])
                nc.vector.tensor_copy(out=dn[32:33, :], in_=a1[64:65, :])
                return dn

            def norm_recip(pp, dn):
                """Batched reciprocal of both denominators on ScalarE as
                exp(-ln(x)) — both functions live in one ACT table set, and
                it keeps the slow iterative divide off DVE. fp16 output so
                the broadcast matmul runs at 1 cycle/row (an fp32 rhs costs
                two half-speed PE passes)."""
                lnt = npool.tile([33, TB], f32, tag="lnt", bufs=2, name=f"lnt{pp}")
                nc.scalar.activation(out=lnt[:], in_=dn[:], func=Ln)
                rcp = npool.tile([33, TB], f16, tag="rcp", bufs=2, name=f"rcp{pp}")
                nc.scalar.activation(out=rcp[:], in_=lnt[:], func=Exp, scale=-1.0)
                return rcp

            def norm_apply(pp, asb, rcp):
                """Both heads at once: one selector matmul broadcasts rcp
                rows 0/32 onto out partitions 0-63/64-127, one DVE multiply
                normalizes the whole pair."""
                rb = aux.tile([128, TB], f32, tag="aux", name=f"rb{pp}")
                nc.tensor.matmul(rb[:], sel[:, :], rcp[:, :])
                nc.vector.tensor_mul(out=att[pp][:, :], in0=asb[:, :], in1=rb[:])

            proj_q(0)
            proj_k(0, 0)
            fps = []  # fc PSUM accumulators; t=0/1 filled in pair 3's loop
            prev = None  # previous pair's SBUF accumulator copies
            prev_dn = None
            prev_rcp = None
            for p in range(NCH):  # head pair p = heads 2p, 2p+1
                a0 = apool.tile([DK + 1, TB], f32, tag="a0", name=f"a0_{p}")
                a1 = apool.tile([DK + 1, TB], f32, tag="a1", name=f"a1_{p}")
                fexp_js = FEXP_J[p]

                def attn_v(j, e):
                    nc.tensor.matmul(
                        a0[:], vpad[j][:, 2 * p, :], e[:, 0:TB],
                        start=(j == 0), stop=(j == NJT - 1),
                    )
                    nc.tensor.matmul(
                        a1[:], vpad[j][:, 2 * p + 1, :], e[:, TB : 2 * TB],
                        start=(j == 0), stop=(j == NJT - 1),
                    )

                pend = None  # (j, e) whose attnV is deferred one iteration
                for j in range(NJT):
                    # pair 0: the rest of K^T, paced with the xt DMA stream
                    # (jb 2/3 need the xt second halves, which land ~23us;
                    # with the compressed head the loop reaches j=6 at
                    # ~27us, so jb2/jb3 sit at j=6/9 to stay behind them)
                    if p == 0 and j in (1, 6, 9):
                        proj_k(0, {1: 1, 6: 2, 9: 3}[j])
                    # next pair's K/Q projection: the early pieces (needed
                    # by its first scores) run mid-pair; the late jb pieces
                    # are emitted at the boundary below as PE filler.
                    # Pair 0 already carries the V projection, so all of
                    # pair 1's pieces move to the boundary instead.
                    if 0 < p < NCH - 1:
                        if j == 10:
                            proj_q(p + 1)
                        elif j in (12, 14):
                            proj_k(p + 1, (j - 12) // 2)
                    # previous pair's normalization, deferred into this
                    # pair's loop so its reciprocal/broadcast work doesn't
                    # gate PE at the boundary. The last pair normalizes
                    # early (j=0-2) so att[2] is ready for in-loop fc
                    # prefill — real PE work that keeps the clock monitor
                    # from throttling the otherwise projection-free pair 3.
                    if prev is not None:
                        if p < NCH - 1:
                            if j == 3:
                                prev_rcp = norm_recip(p - 1, prev_dn)
                            elif j == 5:
                                norm_apply(p - 1, prev, prev_rcp)
                        else:
                            if j == 0:
                                prev_rcp = norm_recip(p - 1, prev_dn)
                            elif j == 1:
                                norm_apply(p - 1, prev, prev_rcp)
                    # fc prefill for token chunks 0/1 interleaved into the
                    # last pair's loop (aux PSUM slots are free here). These
                    # full-array matmuls also re-warm the clock monitor,
                    # which the half-array sc/attnV mix cannot.
                    if p == NCH - 1 and j in (3, 5, 7, 9, 11, 13):
                        t = int(j >= 9)
                        c = ((j - 3) % 6) // 2
                        if c == 0:
                            fp = aux.tile([128, C], f32, tag="aux", name=f"fp{t}")
                            fps.append(fp)
                            nc.tensor.matmul(
                                fp[:], ones_t[0:1, 0:128], bfct[:],
                                start=True, stop=False,
                            )
                        nc.tensor.matmul(
                            fps[t][:], att[c][:, ts(t, 128)], wf[c][:],
                            start=False, stop=False,
                        )
                    sc = scp.tile([128, 2 * TB], f32, tag="sc", name=f"sc{p}_{j}")
                    nc.tensor.matmul(
                        sc[:, 0:TB], kt[p][0:64, ts(j, 128)], qt[p][0:64, :]
                    )
                    nc.tensor.matmul(
                        sc[:, TB : 2 * TB], kt[p][64:128, ts(j, 128)], qt[p][64:128, :]
                    )
                    e = ep.tile([128, 2 * TB], bf16, tag="e", name=f"e{p}_{j}")
                    if j in fexp_js:
                        # Schraudolph fast-exp on DVE: bf16 bit pattern via
                        # int16 round(x*C1 + C2); error ~3% per weight,
                        # washes out over the 2048-key softmax average
                        nc.vector.tensor_scalar(
                            out=e[:].bitcast(i16), in0=sc[:],
                            scalar1=FEXP_C1, scalar2=FEXP_C2, op0=Mult, op1=Add,
                        )
                    else:
                        nc.scalar.activation(out=e[:], in_=sc[:], func=Exp, scale=0.125)
                    # V projection emitted after scores/exp so a late wv/xt
                    # DMA can't block the PE stream ahead of the scores
                    if p == 0:
                        proj_v(j)
                    # attnV deferred one iteration: the tensor queue is
                    # strict FIFO, so attnV(j) right here would block the
                    # already-queued sc(j+1) behind exp(j). One iteration
                    # of skew keeps PE from ever waiting on the exp stream.
                    if pend is not None:
                        attn_v(*pend)
                    pend = (j, e)
                attn_v(*pend)
                # evacuate both accumulators into one [128, TB] SBUF tile
                # (head 2p rows 0-63, head 2p+1 rows 64-127) — frees the
                # PSUM banks and feeds the single-multiply normalization
                asb = npool.tile([128, TB], f32, tag="asb", bufs=2, name=f"asb_{p}")
                nc.vector.tensor_copy(out=asb[0:64, :], in_=a0[0:64, :])
                nc.scalar.copy(out=asb[64:128, :], in_=a1[0:64, :])
                prev = asb
                prev_dn = norm_gather(p, a0, a1)
                # boundary PE filler: the next pair's late K pieces (not
                # needed until its scores j>=8) keep HAM warm through the
                # cross-pair dependency funnel
                if p == 0:
                    proj_q(1)
                    proj_k(1, 0)
                    proj_k(1, 1)
                if p + 1 < NCH:
                    proj_k(p + 1, 2)
                    proj_k(p + 1, 3)

            # ---- tail: all four fc tiles pre-accumulate bias + the first
            # three chunks (PE filler while the last pair's reciprocals run
            # on ScalarE); only the final chunk waits on normalize(3) ----
            def fc_prefill(t, fp):
                nc.tensor.matmul(
                    fp[:], ones_t[0:1, 0:128], bfct[:], start=True, stop=False
                )
                for c in range(NCH - 1):
                    nc.tensor.matmul(
                        fp[:], att[c][:, ts(t, 128)], wf[c][:],
                        start=False, stop=False,
                    )

            for t in (2, 3):
                # scores pool is draining by now; reuse its slots
                fp = scp.tile([128, C], f32, tag="sc", name=f"fp{t}")
                fc_prefill(t, fp)
                fps.append(fp)
            # last pair's normalization, pipelined per fc token chunk: the
            # reciprocal is split into [33, 128] pieces so the first fc
            # final starts ~1us earlier and the whole chunk chain
            # (recip -> broadcast -> multiply -> fc -> evac -> DMA)
            # overlaps across chunks. rb3 lives in the attnV accumulator
            # pool (free after the acopies); each chunk's broadcast writes
            # its own column window of the one PSUM bank.
            rb3 = apool.tile([128, TB], f32, tag="a0", name="rb3")
            out_q = [nc.sync, nc.scalar, nc.gpsimd, nc.sync]
            for t in range(NTT):
                lnt_t = npool.tile([33, 128], f32, tag="lnt", bufs=2, name=f"lnt3_{t}")
                nc.scalar.activation(out=lnt_t[:], in_=prev_dn[:, ts(t, 128)], func=Ln)
                rcp_t = npool.tile([33, 128], f16, tag="rcp", bufs=2, name=f"rcp3_{t}")
                nc.scalar.activation(out=rcp_t[:], in_=lnt_t[:], func=Exp, scale=-1.0)
                nc.tensor.matmul(rb3[:, ts(t, 128)], sel[:, :], rcp_t[:, :])
                nc.vector.tensor_mul(
                    out=att[NCH - 1][:, ts(t, 128)],
                    in0=prev[:, ts(t, 128)],
                    in1=rb3[:, ts(t, 128)],
                )
                nc.tensor.matmul(
                    fps[t][:], att[NCH - 1][:, ts(t, 128)], wf[NCH - 1][:],
                    start=False, stop=True,
                )
                ot = npool.tile([128, C], bf16, tag="ot", bufs=4, name=f"ot{t}")
                if t % 2 == 0:
                    nc.vector.tensor_copy(out=ot[:], in_=fps[t][:])
                else:
                    nc.scalar.copy(out=ot[:], in_=fps[t][:])
                out_q[t].dma_start(out=out_d[ts(t, 128), :], in_=ot[:])

    _split_multi_waits(nc)
    nc.finalize()
    return nc


def get_nc():
    if "nc" not in _CACHE:
        _install_tile_drain_patch()
        _CACHE["nc"] = _build()
    return _CACHE["nc"]


def make_in_maps(x, Wq, bq, Wk, bk, Wv, bv, Wfc, bfc):
    bf = ml_dtypes.bfloat16
    x = np.asarray(x, np.float32)
    Wq, Wk, Wv, Wfc = (np.asarray(w, np.float32) for w in (Wq, Wk, Wv, Wfc))
    bq, bk, bv, bfc = (np.asarray(v, np.float32) for v in (bq, bk, bv, bfc))

    def interleave(wT):
        # [C, cols] -> [128, NCH*cols] with chunk c at columns [c*cols:...]
        cols = wT.shape[1]
        return np.ascontiguousarray(
            wT.reshape(NCH, 128, cols).transpose(1, 0, 2).reshape(128, NCH * cols)
        )

    def interleave_d(wT):
        # [C, C] -> [128, NCH*C] d-major: block (d, c) at columns
        # [(d*NCH+c)*128 : ...], so the d=0 slice [:, 0:NCH*128] ships as
        # one small early DMA
        return np.ascontiguousarray(
            wT.reshape(NCH, 128, NCH, 128).transpose(1, 2, 0, 3).reshape(128, NCH * C)
        )

    bfc_folded = (Wfc @ bv + bfc).reshape(1, C).astype(bf)
    wqT = interleave_d(np.ascontiguousarray(Wq.T).astype(bf))
    wkT = interleave_d(np.ascontiguousarray(Wk.T).astype(bf))
    wvT = interleave(np.ascontiguousarray(Wv.T).astype(bf))
    wfT = interleave(np.ascontiguousarray(Wfc.T).astype(bf))
    bias_c = np.concatenate(
        [bq.reshape(NCH, 128).T, bk.reshape(NCH, 128).T], axis=1
    ).astype(np.float32)

    in_maps = []
    for core in range(N_CORES):
        b, t = divmod(core, N_TOK // TB)
        XT = np.ascontiguousarray(x[b].reshape(N_TOK, C).T).astype(bf)
        # rotate tokens so this core's block sits at columns 0:TB — the Q
        # projection then reads xt directly (attention is j-permutation
        # invariant, so K/V token order doesn't matter as long as kt and
        # vpad agree, which they do: both derive from this xt)
        perm = np.r_[t * TB : (t + 1) * TB, 0 : t * TB, (t + 1) * TB : N_TOK]
        in_maps.append(
            {
                "xt": np.ascontiguousarray(XT[:, perm]),
                "wqT": wqT,
                "wkT": wkT,
                "wvT": wvT,
                "wfT": wfT,
                "bias": bias_c,
                "bfc": bfc_folded,
            }
        )
    return in_maps


def assemble(outs):
    """outs: list of 8 dicts with 'out' (512, 512) -> (2, 512, 64, 32)."""
    per_batch = [
        np.concatenate([outs[b * 4 + t]["out"] for t in range(4)], axis=0)
        for b in range(B)
    ]
    return np.stack(per_batch).reshape(B, C, 64, 32).astype(np.float32)


def kernel(**inputs):
    from concourse.bass_utils import run_bass_kernel_spmd

    nc = get_nc()
    in_maps = make_in_maps(**inputs)
    res = run_bass_kernel_spmd(nc, in_maps, list(range(N_CORES)))
    return assemble(res.results)

